# revision 8
# baseline (speedup 1.0000x reference)
"""Capsule-routing kernel for Trainium2 (8 NeuronCores, data-parallel over batch).

Problem: B=64, IN_CAPS=2048, IN_DIM=64, NUM=32, DIM=64, routings=3.

  hat = einsum('bni,io->bno', inputs, kernel[0]).reshape(B, 32, 2048, 64)
  3 rounds of dynamic routing (softmax over capsules / squash), return o [B,32,64].

Key algebraic compression: hat is NEVER materialized (it is 1 GiB).  With the
row-major reshape, hat_r[n, G*32+g, d] = X[n*64+G, :] @ W[:, g*64+d], so every
routing contraction factors through X [2048,64] and W [64,2048] directly:

  o-step : P_n[ind,g] = sum_G c[n,G,g] * X[n*64+G, ind]          (per-capsule MM)
           o[n,d]     = sum_g  P_all_g[:,n].T @ W[:, g*64:..]    (PSUM-accumulated)
  b-step : Z_g[ind,n] = W[:, g*64:..] @ o[n,:].T                 (per-g MM)
           db[n,G,g]  = X[n*64+G, :] @ Z[:, n, g]                (per-capsule MM)

Routing logits b / coupling c live in a [64(G), 32(n), 32(g)] layout: the
capsule axis n is a FREE dim, so the softmax over capsules is a plain free-dim
reduce and every matmul operand sits at partition base 0 (base-64 operands
trip a hardware error).  All fp32; the small matmuls are LDW/dispatch bound.
"""

import numpy as np

B, IN_CAPS, IN_DIM = 64, 2048, 64
NUM, DIM = 32, 64
N_CORES = 8
BPC = B // N_CORES  # batches per core
EPS = 1e-7

_CACHE = {}


def _build_nc(bpc=BPC, stage=99):
    import concourse.bacc as bacc
    import concourse.tile as tile
    from concourse import mybir

    f32 = mybir.dt.float32
    Act = mybir.ActivationFunctionType
    Alu = mybir.AluOpType

    nc = bacc.Bacc("TRN2", target_bir_lowering=False, debug=False, num_devices=N_CORES)

    # ---- DRAM I/O (per-core shapes) ----
    # x[b, G, n, i] = X[b, n*64+G, i]
    x_d = nc.dram_tensor("x", [bpc, 64, NUM, IN_DIM], f32, kind="ExternalInput")
    # xt2[b, q, r] = X[b, r, q % 64]
    xt2_d = nc.dram_tensor("xt2", [bpc, 128, IN_CAPS], f32, kind="ExternalInput")
    # xs[b, i, n] = sum_G X[b, n*64+G, i]
    xs_d = nc.dram_tensor("xs", [bpc, IN_DIM, NUM], f32, kind="ExternalInput")
    w_d = nc.dram_tensor("w", [IN_DIM, NUM * DIM], f32, kind="ExternalInput")
    # wt[d, g, i] = W[i, g*64+d]
    wt_d = nc.dram_tensor("wt", [IN_DIM, 32, IN_DIM], f32, kind="ExternalInput")
    wsum_d = nc.dram_tensor("wsum", [IN_DIM, DIM], f32, kind="ExternalInput")
    i32_d = nc.dram_tensor("i32", [32, 32], f32, kind="ExternalInput")
    out_d = nc.dram_tensor("out", [bpc, NUM, DIM], f32, kind="ExternalOutput")

    with tile.TileContext(nc) as tc:
        with (
            tc.tile_pool(name="const", bufs=1) as cpool,
            tc.tile_pool(name="inp", bufs=2) as ipool,
            tc.tile_pool(name="work", bufs=2) as wpool,
            tc.tile_pool(name="ps_big", bufs=2, space="PSUM") as ps_big,
            tc.tile_pool(name="ps_acc", bufs=1, space="PSUM") as ps_acc,
            tc.tile_pool(name="ps_o", bufs=1, space="PSUM") as ps_o,
            tc.tile_pool(name="ps_sm", bufs=1, space="PSUM") as ps_sm,
        ):
            # ---- constants to SBUF ----
            w_t = cpool.tile([IN_DIM, NUM * DIM], f32, tag="w")
            nc.sync.dma_start(w_t[:], w_d[:])
            wt_t = cpool.tile([IN_DIM, 32, IN_DIM], f32, tag="wt")
            nc.sync.dma_start(wt_t[:], wt_d[:])
            wsum_t = cpool.tile([IN_DIM, DIM], f32, tag="wsum")
            nc.sync.dma_start(wsum_t[:], wsum_d[:])
            i32_t = cpool.tile([32, 32], f32, tag="i32")
            nc.sync.dma_start(i32_t[:], i32_d[:])

            def squash(o_ps):
                """psum [32,64] -> sbuf [32,64] squashed."""
                o_sb = wpool.tile([NUM, DIM], f32, tag="osb")
                nc.vector.tensor_copy(o_sb[:], o_ps[:])
                o2 = wpool.tile([NUM, DIM], f32, tag="o2")
                s0 = wpool.tile([NUM, 1], f32, tag="s0")
                nc.scalar.activation(o2[:], o_ps[:], Act.Square, accum_out=s0[:])
                s = wpool.tile([NUM, 1], f32, tag="s")
                nc.vector.tensor_scalar_add(s[:], s0[:], EPS)  # s = sum(o^2) + EPS
                u = wpool.tile([NUM, 1], f32, tag="u")
                nc.scalar.activation(u[:], s[:], Act.Sqrt)
                v = wpool.tile([NUM, 1], f32, tag="v")
                nc.vector.tensor_scalar_add(v[:], s[:], 1.0)
                rv = wpool.tile([NUM, 1], f32, tag="rv")
                nc.vector.reciprocal(rv[:], v[:])
                f = wpool.tile([NUM, 1], f32, tag="f")
                nc.vector.tensor_mul(f[:], u[:], rv[:])
                o_sq = wpool.tile([NUM, DIM], f32, tag="osq")
                nc.vector.tensor_scalar_mul(o_sq[:], o_sb[:], f[:])
                return o_sq

            def transpose_o(o_sq):
                """sbuf [32,64] -> sbuf [64,32] (o^T)."""
                t_ps = ps_sm.tile([128, NUM], f32, tag="sm")
                nc.tensor.transpose(t_ps[0:64, :], o_sq[:], i32_t[:])
                oT = wpool.tile([IN_DIM, NUM], f32, tag="oT")
                nc.any.tensor_copy(oT[:], t_ps[0:64, :])
                return oT

            def bchain(oT, xt2_t):
                """-> db psum [64,32,32]: db[G, n, g] = o_n . hat_r[n, G*32+g, :]"""
                z_ps = ps_big.tile([IN_DIM, 32, 32], f32, tag="big")  # [ind, g, n]
                for g in range(32):
                    nc.tensor.matmul(
                        z_ps[:, g, :], lhsT=wt_t[:, g, :], rhs=oT[:],
                        start=True, stop=True,
                    )
                z_sb = wpool.tile([IN_DIM, 32, 32], f32, tag="zp")
                nc.any.tensor_copy(z_sb[:], z_ps[:])
                db_ps = ps_acc.tile([64, NUM, 32], f32, tag="db")  # [G, n, g]
                for n in range(32):
                    nc.tensor.matmul(
                        db_ps[:, n, :],
                        lhsT=xt2_t[0:64, n * 64 : (n + 1) * 64],
                        rhs=z_sb[:, :, n],
                        start=True, stop=True,
                    )
                return db_ps

            def softmax(b_sb):
                """sbuf b [64,32n,32g] -> sbuf c, softmax over the n axis."""
                expb = wpool.tile([64, NUM, 32], f32, tag="expb")
                nc.scalar.activation(expb[:], b_sb[:], Act.Exp)
                S = wpool.tile([64, 32], f32, tag="S")  # [G, g]
                nc.vector.tensor_reduce(
                    S[:], expb[:].transpose([0, 2, 1]), mybir.AxisListType.X, Alu.add
                )
                rs = wpool.tile([64, 32], f32, tag="rs")
                nc.vector.reciprocal(rs[:], S[:])
                c_sb = wpool.tile([64, NUM, 32], f32, tag="c")
                nc.vector.tensor_mul(
                    c_sb[:], expb[:], rs[:, None, :].to_broadcast([64, NUM, 32])
                )
                return c_sb

            def ochain(c_sb, x_t, o_ps):
                p_ps = ps_big.tile([IN_DIM, 32, 32], f32, tag="big")  # [ind, n, g]
                for n in range(32):
                    nc.tensor.matmul(
                        p_ps[:, n, :], lhsT=x_t[:, n, :], rhs=c_sb[:, n, :],
                        start=True, stop=True,
                    )
                p_sb = wpool.tile([IN_DIM, 32, 32], f32, tag="zp")
                nc.any.tensor_copy(p_sb[:], p_ps[:])
                for g in range(32):
                    nc.tensor.matmul(
                        o_ps[:],
                        lhsT=p_sb[:, :, g],
                        rhs=w_t[:, g * 64 : (g + 1) * 64],
                        start=(g == 0),
                        stop=(g == 31),
                    )

            # ---- per-batch body, fully unrolled ----
            for b in range(bpc):
                x_t = ipool.tile([64, NUM, IN_DIM], f32, tag="x")
                nc.sync.dma_start(x_t[:], x_d[b])
                xt2_t = ipool.tile([128, IN_CAPS], f32, tag="xt2")
                nc.sync.dma_start(xt2_t[:], xt2_d[b])
                xs_t = ipool.tile([IN_DIM, NUM], f32, tag="xs")
                nc.sync.dma_start(xs_t[:], xs_d[b])

                # iteration 0: c is uniform 1/32 -> single matmul
                o_ps = ps_o.tile([NUM, DIM], f32, tag="o")
                nc.tensor.matmul(o_ps[:], lhsT=xs_t[:], rhs=wsum_t[:], start=True, stop=True)
                o_sq = squash(o_ps)

                b_sb = None
                for it in range(2):
                    oT = transpose_o(o_sq)
                    db_ps = bchain(oT, xt2_t)
                    nb = wpool.tile([64, NUM, 32], f32, tag="b")
                    if it == 0:
                        nc.vector.tensor_copy(nb[:], db_ps[:])
                    else:
                        nc.vector.tensor_add(nb[:], b_sb[:], db_ps[:])
                    b_sb = nb
                    c_sb = softmax(b_sb)
                    o_ps = ps_o.tile([NUM, DIM], f32, tag="o")
                    ochain(c_sb, x_t, o_ps)
                    o_sq = squash(o_ps)

                nc.sync.dma_start(out_d[b], o_sq[:])

    nc.compile()
    return nc


def _get_nc():
    if "nc" not in _CACHE:
        _CACHE["nc"] = _build_nc()
    return _CACHE["nc"]


def _prep_host_small(inputs, kern):
    """Host-side input prep for arbitrary leading batch size."""
    Bn = inputs.shape[0]
    X = np.ascontiguousarray(inputs, dtype=np.float32)
    W = np.ascontiguousarray(kern.reshape(IN_DIM, NUM * DIM), dtype=np.float32)
    # x[b, G, n, i] = X[b, n*64+G, i]
    x_h = np.ascontiguousarray(X.reshape(Bn, NUM, 64, IN_DIM).transpose(0, 2, 1, 3))
    xt = X.transpose(0, 2, 1)  # [B, 64, 2048]
    xt2_h = np.ascontiguousarray(np.concatenate([xt, xt], axis=1))  # [B, 128, 2048]
    xs_h = np.ascontiguousarray(X.reshape(Bn, NUM, 64, IN_DIM).sum(axis=2).transpose(0, 2, 1))
    # wt[d, g, i] = W[i, g*64 + d]
    wt_h = np.ascontiguousarray(W.reshape(IN_DIM, 32, 64).transpose(2, 1, 0))
    wsum_h = np.ascontiguousarray(W.reshape(IN_DIM, 32, 64).sum(axis=1) / 32.0)
    i32_h = np.eye(32, dtype=np.float32)
    return x_h, xt2_h, xs_h, W, wt_h, wsum_h, i32_h


def _prep_host(inputs, kern):
    return _prep_host_small(inputs, kern)


def _make_in_maps(inputs, kern):
    x_h, xt2_h, xs_h, W, wt_h, wsum_h, i32_h = _prep_host_small(
        np.asarray(inputs), np.asarray(kern)
    )
    in_maps = []
    for c in range(N_CORES):
        sl = slice(c * BPC, (c + 1) * BPC)
        in_maps.append(
            {
                "x": x_h[sl],
                "xt2": xt2_h[sl],
                "xs": xs_h[sl],
                "w": W,
                "wt": wt_h,
                "wsum": wsum_h,
                "i32": i32_h,
            }
        )
    return in_maps


def kernel(inputs, kernel, num_capsule=NUM, dim_capsule=DIM, routings=3, **_):
    from concourse.bass_utils import run_bass_kernel_spmd

    assert int(num_capsule) == NUM and int(dim_capsule) == DIM and int(routings) == 3
    nc = _get_nc()
    in_maps = _make_in_maps(inputs, kernel)
    res = run_bass_kernel_spmd(nc, in_maps, core_ids=list(range(N_CORES)))
    out = np.concatenate([res.results[c]["out"] for c in range(N_CORES)], axis=0)
    return out.astype(np.float32)
